# revision 60
# baseline (speedup 1.0000x reference)
"""Trainium2 Bass kernel for spherical deep GMM classifier (DGMMC).

Reference computation (B=8192, D=1024, C=128 classes, K=8 comps, N=C*K=1024):
    bw = clip(bandwidths, 1e-3, 100); a = 1/bw
    log_prob[b,n] = -0.5*(D*log(2pi) + D*log(bw[n]) + sq_dist[b,n]/bw[n])
    log_prob += log_softmax(weights.reshape(C,K),1).reshape(N)
    lse1[b,c]  = LSE_k(log_prob[b,c*K+k]) + log_softmax(priors)[c]
    out[b,c]   = lse1[b,c] - LSE_c(lse1[b,c])

Strategy: data-parallel over batch across 8 cores.  The device computes the
single dominant GEMM  dot[b,n] = x[b,:] @ (means * (1/bw))[n,:].T  and writes
dot back as fp16; 6 of the 8 128-dim contraction subtiles run as fp8-e4m3
DoubleRow matmuls, the other 2 as fp16 (full fp8 fails the 2e-2 gate:
2.07e-2 measured offline).  Everything else is O(B*N) or smaller and runs
on the host around the device call.

Perf structure (vs. the naive schedule):
  - Input DMA triggers are hoisted into the kernel-entry all-engine
    barrier's "arrive" slots on SP/ACT, so HBM traffic starts ~2.5us
    earlier (right after the fixed walrus engine preamble instead of after
    the const-init barrier).
  - A chain of dummy "warmup" matmuls reading uninitialized SBUF is
    hoisted the same way onto the PE queue: the HAM clock-ramp (~4us of
    sustained PE activity to reach 8/8) runs concurrently with the input
    load instead of serializing in front of the real stream.
  - Each means chunk is split into its two 512-column halves with one half
    on each HWDGE queue, so both queues feed the first b-tile's pass sweep
    concurrently (~330 GB/s aggregate).
  - x stripes ride the gpsimd software-DGE queue, keeping the two HWDGE
    queues free for means + output traffic.
  - The last b-tile is processed h-chunk-major and its output is cast +
    DMA'd per 512-column chunk, split across both queues by partition
    halves, to minimize the post-stream tail.
"""

import math

import numpy as np

B, D, C, K = 8192, 1024, 128, 8
N = C * K
NCORES = 8
BLOC = B // NCORES  # rows per core
P = 128
NSUB = D // P  # 128-row contraction subtiles
NH = N // 512
LOG_2PI = math.log(2.0 * math.pi)

_CACHE: dict = {}


def _build_nc(cfg=None):
    import concourse.bacc as bacc
    import concourse.bass as bass
    import concourse.mybir as mybir
    import concourse.tile as tile
    from concourse.tile import add_dep_helper

    defaults = dict(
        fp8_sub=6,       # leading 128-dim subtiles done in e4m3 DoubleRow
        warmup=12,       # dummy matmuls (garbage SBUF src) hoisted into the
                         # entry barrier to ramp the HAM clock to 8/8 while
                         # the input DMA head is in flight
        warmup_tail=1,   # non-hoisted warmups at stream top as a cushion so
                         # the PE never gaps between warmup chain and data
        hoist=True,      # move triggers/warmups into block-0 arrive slots
        xt_queue="pool", # "pool": x stripes on gpsimd SWDGE; "hw": on HWDGE
        bufs_work=4,
        psum_bufs=3,     # 3x [P,N] fp32 = 6 banks; wu_ps takes a 7th
        sem_stop=200,    # shrink kernel semaphore range (reset-sweep length)
        tail_stagger=True,
        tail_split=True, # split last-tile output DMAs across both queues
        mt_chunks="halfall",  # "pair": ~128KB mt DMAs; "halfall": one per half
        dummy_flush=False,  # tiny DMA after each mt load to flush its sem
        mid_order="h",   # bt>=2 matmul order: "h" chunk-major / "pass" major
        out_split=True,  # split non-tail outputs across both queues
        pace_in=0,       # input-queue depth limit (0 = unpaced): trigger k
                         # waits DMA k-pace_in, so completion sems fire
                         # promptly after their data instead of lagging a
                         # whole queued DMA behind
    )
    cfg = {**defaults, **(cfg or {})}

    f32 = mybir.dt.float32
    f16 = mybir.dt.float16
    f8 = mybir.dt.float8e4
    DR = mybir.MatmulPerfMode.DoubleRow
    ET = mybir.EngineType

    S8 = cfg["fp8_sub"]
    assert S8 % 2 == 0 and 0 <= S8 <= NSUB
    S16 = NSUB - S8
    NPAIR = S8 // 2

    orig_range = bass.get_kernel_semaphore_range
    if cfg["sem_stop"]:
        stop = cfg["sem_stop"]
        bass.get_kernel_semaphore_range = lambda: range(150, stop)
    try:
        nc = bacc.Bacc(None, target_bir_lowering=False)
    finally:
        bass.get_kernel_semaphore_range = orig_range

    NB = BLOC // P  # number of 128-row batch tiles per core

    # x is host-prepacked into per-b-tile stripes already in SBUF layout
    # [bt, p, bytes] with the fp8 subtiles' bytes followed by the fp16
    # subtiles' bytes, so each b-tile's stationary operand is one contiguous
    # full-bandwidth DMA; matmuls use bitcast views into the stripe.
    X8B = S8 * P  # fp8 bytes per stripe row
    XBYTES = X8B + S16 * P * 2
    u8 = mybir.dt.uint8
    # partition-major so multi-stripe slices match the SBUF layout directly
    xtc = nc.dram_tensor("xtc", [P, NB, XBYTES], u8, kind="ExternalInput")
    mt8h = mt16h = None
    if S8:
        # [half, p, s, 512]: "pair" mode slices [h, :, 2c:2c+2, :] (~128KB
        # DMAs, 1KB rows), "halfall" mode loads a whole half [h] as one DMA
        # with 3KB-contiguous rows
        mt8h = nc.dram_tensor("mt8h", [NH, P, S8, 512], f8, kind="ExternalInput")
    if S16:
        mt16h = nc.dram_tensor("mt16h", [NH, P, S16, 512], f16, kind="ExternalInput")
    lp = nc.dram_tensor("lp", [BLOC, N], f16, kind="ExternalOutput")

    # dummy warmup operands: raw (non-tile) SBUF garbage + a dedicated PSUM
    # bank; no data deps, so the hoisted matmuls are free of semaphore waits.
    # matmul() auto-emits a paired InstLdweights, so capture the warmup
    # instructions by diffing the entry block around the emission.
    wu_src = nc.alloc_sbuf_tensor("wu_src", [P, 512], f16)
    wu_ps = nc.alloc_psum_tensor("wu_ps", [P, 512], f32)
    mainblk = nc.m.functions[0].blocks[0]
    n_before = len(mainblk.instructions)
    for _ in range(cfg["warmup"]):
        nc.tensor.matmul(
            wu_ps[:], wu_src[:, 0:P], wu_src[:], start=True, stop=True
        )
    wu_insts = list(mainblk.instructions[n_before:])

    trig = {"sync": [], "scalar": [], "pool": []}

    with tile.TileContext(nc) as tc:
        with (
            tc.tile_pool(name="resident", bufs=1) as resident,
            tc.tile_pool(name="work", bufs=cfg["bufs_work"]) as work,
            tc.tile_pool(name="psum", bufs=cfg["psum_bufs"], space="PSUM") as psum_pool,
        ):
            xtc_sb = resident.tile([P, NB, XBYTES], u8, name="xtc_sb")
            # h-major SBUF layout: chunk (c|s, h) is contiguous within each
            # partition, so its load is one >=1KB-row DMA per partition
            mt8_sb = (
                resident.tile([P, NH, S8, 512], f8, name="mt8_sb") if S8 else None
            )
            mt16_sb = (
                resident.tile([P, NH, S16, 512], f16, name="mt16_sb") if S16 else None
            )

            def x8_view(bt, c):  # DR pass c stationary operand [P, 2, 128]
                return (
                    xtc_sb[:, bt, 0:X8B]
                    .bitcast(f8)
                    .rearrange("p (s c) -> p s c", c=P)[:, 2 * c : 2 * c + 2, :]
                )

            def x16_view(bt, s):  # fp16 pass s stationary operand [P, 128]
                return (
                    xtc_sb[:, bt, X8B:XBYTES]
                    .bitcast(f16)
                    .rearrange("p (s c) -> p s c", c=P)[:, s, :]
                )

            # extra warmups that stay at the top of the PE stream: if the
            # input DMA head outlasts the hoisted warmup chain these bridge
            # the gap so HAM doesn't re-throttle
            for _ in range(cfg["warmup_tail"]):
                nc.tensor.matmul(
                    wu_ps[:], wu_src[:, 0:P], wu_src[:], start=True, stop=True
                )

            # Input loads.  The 16 DMA data movers are a pool shared across
            # queues, so concurrent queues delay each other's COMPLETIONS
            # (a chunk's semaphore only fires when its last packet drains).
            # Instead, all critical inputs go down ONE queue as a few large
            # consolidated DMAs in exact consumption order: x stripes 0+1,
            # then each column-half's means (fp8 then fp16).  The remaining
            # x stripes ride the gpsimd SWDGE queue, deferred until the
            # critical loads have landed.
            seq = [(xtc_sb[:, 0:2], xtc[:, 0:2])]
            for h in range(NH):
                if cfg["mt_chunks"] == "pair":
                    for c in range(NPAIR):
                        seq.append(
                            (
                                mt8_sb[:, h, 2 * c : 2 * c + 2, :],
                                mt8h[h, :, 2 * c : 2 * c + 2, :],
                            )
                        )
                else:
                    seq.append((mt8_sb[:, h], mt8h[h]))
                if S16:
                    seq.append((mt16_sb[:, h], mt16h[h]))
            flush_sc = None
            if cfg["dummy_flush"]:
                flush_sc = nc.alloc_sbuf_tensor("flush_sc", [P, 64], u8)
            last_in = None
            in_tis = []
            for dst, src in seq:
                ti = nc.sync.dma_start(dst, src)
                if cfg["pace_in"] and len(in_tis) >= cfg["pace_in"]:
                    add_dep_helper(
                        ti.ins,
                        in_tis[-cfg["pace_in"]].ins,
                        sync=True,
                        reason="bound input queue depth",
                    )
                in_tis.append(ti)
                last_in = ti
                trig["sync"].append(ti.ins)
                if flush_sc is not None:
                    # tiny follow-up DMA so the big one's completion
                    # semaphore isn't held back by queue lookahead
                    fi = nc.sync.dma_start(flush_sc[:], xtc[:, 0, 0:64])
                    trig["sync"].append(fi.ins)
            xt_eng = {"pool": nc.gpsimd, "sync": nc.sync, "scalar": nc.scalar}[
                cfg["xt_queue"]
            ]
            xt_key = {"pool": "pool", "sync": "sync", "scalar": "scalar"}[
                cfg["xt_queue"]
            ]
            for bt in range(2, NB):
                ti = xt_eng.dma_start(xtc_sb[:, bt], xtc[:, bt])
                if bt == 2 and xt_key == "pool":
                    add_dep_helper(
                        ti.ins,
                        last_in.ins,
                        sync=True,
                        reason="defer SWDGE xt behind critical HWDGE loads",
                    )
                trig[xt_key].append(ti.ins)

            # matmul pass list: DoubleRow fp8 pairs first, fp16 after
            def passes(bt):
                out = []
                for c in range(NPAIR):
                    out.append(
                        lambda h, c=c, bt=bt: (
                            x8_view(bt, c),
                            mt8_sb[:, h, 2 * c : 2 * c + 2, :],
                            DR,
                        )
                    )
                for s in range(S16):
                    out.append(
                        lambda h, s=s, bt=bt: (
                            x16_view(bt, s),
                            mt16_sb[:, h, s, :],
                            None,
                        )
                    )
                return out

            prev_mm = None

            def emit_out(bt, ot, h=None, split=True):
                # split outputs across both HWDGE queues by partition
                # halves so no single queue ever backs up at the tail
                half = P // 2
                csl = slice(0, N) if h is None else slice(h * 512, (h + 1) * 512)
                if split:
                    nc.sync.dma_start(lp[bt * P : bt * P + half, csl], ot[0:half])
                    nc.scalar.dma_start(
                        lp[bt * P + half : (bt + 1) * P, csl], ot[half:P]
                    )
                else:
                    [nc.sync, nc.scalar][bt % 2].dma_start(
                        lp[bt * P : (bt + 1) * P, csl], ot[0:P]
                    )

            # b-tiles 0+1 run as one fused pass-major group: the mt chunk
            # demand rate is halved (one chunk per 2 matmuls) while the
            # input DMAs are still landing, so the PE never gaps (a gap
            # >~1us makes HAM re-throttle the clock to 4/8 mid-stream)
            ps01 = [
                psum_pool.tile([P, N], f32, tag="ps", name=f"ps0{b}")
                for b in range(2)
            ]
            pl01 = [passes(0), passes(1)]
            npass = len(pl01[0])
            for h in range(NH):
                for i in range(npass):
                    for b in range(2):
                        lhsT, rhs, pm = pl01[b][i](h)
                        mmi = nc.tensor.matmul(
                            ps01[b][:, h * 512 : (h + 1) * 512],
                            lhsT,
                            rhs,
                            start=(i == 0),
                            stop=(i == npass - 1),
                            perf_mode=pm,
                        )
                        if i == npass - 1 and b == 1:
                            prev_mm = mmi
            for b in range(2):
                ot = work.tile([P, N], f16, tag="ot", name=f"ot0{b}")
                nc.vector.tensor_copy(ot, ps01[b])
                emit_out(b, ot, split=cfg["out_split"])

            for bt in range(2, NB):
                stag = cfg["tail_stagger"] and bt == NB - 1
                if stag:
                    # separate per-h psum tiles so h0's cast (a tile-level
                    # read) doesn't falsely serialize against h1's matmuls
                    ps_h = [
                        psum_pool.tile([P, 512], f32, tag="ps", name=f"pst{h}")
                        for h in range(NH)
                    ]
                else:
                    ps = psum_pool.tile([P, N], f32, tag="ps")
                pl = passes(bt)
                npass = len(pl)
                # h-chunk-major on the last tile so h0's cast+DMA overlap
                # h1's matmuls; optionally pass-major mid-stream (fewer
                # accumulation-group boundary stalls).  NOTE: pass order
                # within an accumulation group must keep all DoubleRow
                # passes before all fp16 passes — mixing them
                # non-monotonically corrupts the accumulation.
                if stag or cfg["mid_order"] == "h":
                    his = [(h, i) for h in range(NH) for i in range(npass)]
                else:
                    his = [(h, i) for i in range(npass) for h in range(NH)]
                for h, i in his:
                    if True:
                        lhsT, rhs, pm = pl[i](h)
                        pdst = (
                            ps_h[h][:, :] if stag else ps[:, h * 512 : (h + 1) * 512]
                        )
                        mmi = nc.tensor.matmul(
                            pdst,
                            lhsT,
                            rhs,
                            start=(i == 0),
                            stop=(i == npass - 1),
                            perf_mode=pm,
                        )
                        # serialize b-tile groups on PE so each group
                        # completes (and its copy-out starts) ASAP
                        if h == 0 and i == 0 and prev_mm is not None:
                            add_dep_helper(
                                mmi.ins,
                                prev_mm.ins,
                                sync=False,
                                reason="group-sequential PE order",
                            )
                        if i == npass - 1:
                            prev_mm = mmi
                            if stag:
                                ot = work.tile([P, 512], f16, tag="ot2")
                                nc.vector.tensor_copy(ot, ps_h[h][:, :])
                                emit_out(bt, ot, h=h, split=cfg["tail_split"])
                if not stag:
                    ot = work.tile([P, N], f16, tag="ot")
                    nc.vector.tensor_copy(ot, ps)
                    emit_out(bt, ot, split=cfg["out_split"])

    if cfg["hoist"]:
        _hoist(nc, mybir, trig, wu_insts)

    nc.compile()
    return nc


def _hoist(nc, mybir, trig, wu_insts):
    """Move the input DMA triggers and warmup matmuls from the tile-context
    block into the entry block's all-engine-barrier arrive slots.

    The entry barrier is, per engine, (InstDrain[arrive], InstEventSemaphore
    [wait-release]); instructions placed between the two run right after that
    engine's fixed walrus preamble without delaying any other engine.  The
    gpsimd x-stripe triggers go after the barrier release (gpsimd is the
    barrier master, so anything before its release EventSemaphore would
    stall every engine)."""
    ET = mybir.EngineType
    f = nc.m.functions[0]
    b0, b1 = f.blocks[0], f.blocks[1]

    moved = {
        ET.SP: list(trig["sync"]),
        ET.Activation: list(trig["scalar"]),
        ET.PE: list(wu_insts),
        ET.Pool: list(trig["pool"]),
    }
    # warmups emitted pre-tile-context already live in b0 (after the
    # barrier); everything else is in b1
    move_ids = {id(x) for insts in moved.values() for x in insts}
    b0.instructions = [x for x in b0.instructions if id(x) not in move_ids]
    b1.instructions = [x for x in b1.instructions if id(x) not in move_ids]

    def arrive_slot(eng):
        for i, ins in enumerate(b0.instructions):
            if isinstance(ins, mybir.InstDrain) and ins.engine == eng:
                return i + 1
        raise RuntimeError(f"no barrier drain found for {eng}")

    def after_release():
        last = None
        for i, ins in enumerate(b0.instructions):
            if isinstance(ins, mybir.InstEventSemaphore) and ins.engine == ET.Pool:
                last = i
        assert last is not None
        return last + 1

    for eng in (ET.SP, ET.Activation, ET.PE):
        if moved[eng]:
            pos = arrive_slot(eng)
            b0.instructions[pos:pos] = moved[eng]
    if moved[ET.Pool]:
        pos = after_release()
        b0.instructions[pos:pos] = moved[ET.Pool]


def _host_prep(x, means, bandwidths, weights, priors, fp8_sub):
    """Pack transposed GEMM operands; compute host-side affine terms."""
    import ml_dtypes

    x = np.asarray(x, dtype=np.float32)
    means = np.asarray(means, dtype=np.float32)

    bw = np.clip(np.asarray(bandwidths, dtype=np.float64), 0.001, 100.0)
    a = 1.0 / bw
    m_sq = np.einsum(
        "nd,nd->n", means.astype(np.float64), means.astype(np.float64)
    )
    w = np.asarray(weights, dtype=np.float64).reshape(C, K)
    log_w = (
        w
        - np.log(np.exp(w - w.max(1, keepdims=True)).sum(1, keepdims=True))
        - w.max(1, keepdims=True)
    ).reshape(N)
    pr = np.asarray(priors, dtype=np.float64)
    log_pri = pr - (np.log(np.exp(pr - pr.max()).sum()) + pr.max())
    cvec = (
        -0.5 * (D * LOG_2PI + D * np.log(bw) + m_sq * a)
        + log_w
        + np.repeat(log_pri, K)
    )
    ah = -0.5 * a
    xsq = np.einsum("bd,bd->b", x.astype(np.float64), x.astype(np.float64))

    # pack x into per-core, per-b-tile stripes [core, bt, p(row), bytes]:
    # fp8 subtile bytes then fp16 subtile bytes, matching the device bitcast
    nbt = BLOC // P
    ds = fp8_sub * P
    xt_t = x.T  # [D, B]
    mt_t = means.T * a  # [D, N]

    def pack_x(arr, dt):  # arr [d, B] -> [core, p(row), bt, sub*col] bytes
        sub = arr.shape[0] // P
        packed = np.ascontiguousarray(
            arr.astype(dt).reshape(sub, P, NCORES, nbt, P).transpose(2, 1, 3, 0, 4)
        )
        return packed.reshape(NCORES, P, nbt, -1).view(np.uint8)

    chunks = []
    if fp8_sub:
        chunks.append(pack_x(xt_t[:ds], ml_dtypes.float8_e4m3))
    if ds < D:
        chunks.append(pack_x(xt_t[ds:], np.float16))
    parts = {"xtc": np.concatenate(chunks, axis=3)}
    if fp8_sub:
        # [h, p, s, j]: element = mt[s*P+p, h*512+j]
        m8 = mt_t[:ds].astype(ml_dtypes.float8_e4m3)
        m8 = m8.reshape(fp8_sub, P, 2, 512).transpose(2, 1, 0, 3)
        parts["mt8h"] = np.ascontiguousarray(m8)
    if ds < D:
        m16 = mt_t[ds:].astype(np.float16)
        m16 = m16.reshape((D - ds) // P, P, 2, 512).transpose(2, 1, 0, 3)
        parts["mt16h"] = np.ascontiguousarray(m16)
    return parts, cvec, ah, xsq


def _host_finish(lp, cvec, ah, xsq):
    """lp: [B, N] fp16 device GEMM result -> [B, C] float32 log-mixture."""
    logp = lp.astype(np.float32)
    logp += cvec.astype(np.float32)[None, :]
    logp += xsq.astype(np.float32)[:, None] * ah.astype(np.float32)[None, :]
    v = logp.reshape(B, C, K)
    m = v.max(2)
    lse1 = m + np.log(np.exp(v - m[:, :, None]).sum(2, dtype=np.float32))
    z = lse1.max(1, keepdims=True)
    out = lse1 - (
        z + np.log(np.exp(lse1 - z).sum(1, keepdims=True, dtype=np.float32))
    )
    return out.astype(np.float32)


def _run(x, means, bandwidths, weights, priors, trace=False, cfg=None):
    from concourse.bass_utils import run_bass_kernel_spmd

    key = tuple(sorted((cfg or {}).items()))
    if key not in _CACHE:
        try:
            _CACHE[key] = _build_nc(cfg)
        except Exception:
            # the entry-block hoisting surgery is purely a perf
            # transformation; fall back to the plain schedule if the
            # framework's block layout ever changes underneath it
            if (cfg or {}).get("hoist", True):
                _CACHE[key] = _build_nc({**(cfg or {}), "hoist": False})
            else:
                raise
    nc = _CACHE[key]
    fp8_sub = (cfg or {}).get("fp8_sub", 6)

    parts, cvec, ah, xsq = _host_prep(
        x, means, bandwidths, weights, priors, fp8_sub
    )
    in_maps = [
        {
            k: np.ascontiguousarray(v[i]) if k.startswith("xt") else v
            for k, v in parts.items()
        }
        for i in range(NCORES)
    ]
    res = run_bass_kernel_spmd(nc, in_maps, core_ids=list(range(NCORES)), trace=trace)
    lp = np.concatenate([r["lp"] for r in res.results], axis=0)
    out = _host_finish(lp, cvec, ah, xsq)
    return out, res


def kernel(x, means, bandwidths, weights, priors):
    out, _ = _run(x, means, bandwidths, weights, priors, trace=False)
    return out


# revision 63
# speedup vs baseline: 1.0556x; 1.0556x over previous
"""Trainium2 Bass kernel for spherical deep GMM classifier (DGMMC).

Reference computation (B=8192, D=1024, C=128 classes, K=8 comps, N=C*K=1024):
    bw = clip(bandwidths, 1e-3, 100); a = 1/bw
    log_prob[b,n] = -0.5*(D*log(2pi) + D*log(bw[n]) + sq_dist[b,n]/bw[n])
    log_prob += log_softmax(weights.reshape(C,K),1).reshape(N)
    lse1[b,c]  = LSE_k(log_prob[b,c*K+k]) + log_softmax(priors)[c]
    out[b,c]   = lse1[b,c] - LSE_c(lse1[b,c])

Strategy: data-parallel over batch across 8 cores.  The device computes the
single dominant GEMM  dot[b,n] = x[b,:] @ (means * (1/bw))[n,:].T  and writes
dot back as fp16; 6 of the 8 128-dim contraction subtiles run as fp8-e4m3
DoubleRow matmuls, the other 2 as fp16 (full fp8 fails the 2e-2 gate:
2.07e-2 measured offline).  Everything else is O(B*N) or smaller and runs
on the host around the device call.

Perf structure (vs. the naive schedule):
  - Input DMA triggers are hoisted into the kernel-entry all-engine
    barrier's "arrive" slots on SP/ACT, so HBM traffic starts ~2.5us
    earlier (right after the fixed walrus engine preamble instead of after
    the const-init barrier).
  - A chain of dummy "warmup" matmuls reading uninitialized SBUF is
    hoisted the same way onto the PE queue: the HAM clock-ramp (~4us of
    sustained PE activity to reach 8/8) runs concurrently with the input
    load instead of serializing in front of the real stream.
  - Each means chunk is split into its two 512-column halves with one half
    on each HWDGE queue, so both queues feed the first b-tile's pass sweep
    concurrently (~330 GB/s aggregate).
  - x stripes ride the gpsimd software-DGE queue, keeping the two HWDGE
    queues free for means + output traffic.
  - The last b-tile is processed h-chunk-major and its output is cast +
    DMA'd per 512-column chunk, split across both queues by partition
    halves, to minimize the post-stream tail.
"""

import math

import numpy as np

B, D, C, K = 8192, 1024, 128, 8
N = C * K
NCORES = 8
BLOC = B // NCORES  # rows per core
P = 128
NSUB = D // P  # 128-row contraction subtiles
NH = N // 512
LOG_2PI = math.log(2.0 * math.pi)

_CACHE: dict = {}


def _build_nc(cfg=None):
    import concourse.bacc as bacc
    import concourse.bass as bass
    import concourse.mybir as mybir
    import concourse.tile as tile
    from concourse.tile import add_dep_helper

    defaults = dict(
        fp8_sub=6,       # leading 128-dim subtiles done in e4m3 DoubleRow
        warmup=12,       # dummy matmuls (garbage SBUF src) hoisted into the
                         # entry barrier to ramp the HAM clock to 8/8 while
                         # the input DMA head is in flight
        warmup_tail=1,   # non-hoisted warmups at stream top as a cushion so
                         # the PE never gaps between warmup chain and data
        hoist=True,      # move triggers/warmups into block-0 arrive slots
        xt_queue="pool", # "pool": x stripes on gpsimd SWDGE; "hw": on HWDGE
        bufs_work=4,
        psum_bufs=3,     # 3x [P,N] fp32 = 6 banks; wu_ps takes a 7th
        sem_stop=200,    # shrink kernel semaphore range (reset-sweep length)
        tail_stagger=True,
        tail_split=True, # split last-tile output DMAs across both queues
        mt_chunks="halfall",  # "pair": ~128KB mt DMAs; "halfall": one per half
        dummy_flush=False,  # tiny DMA after each mt load to flush its sem
        mid_order="h",   # bt>=2 matmul order: "h" chunk-major / "pass" major
        out_split=False,  # single alternating-queue DMA per non-tail output
        pace_in=0,       # input-queue depth limit (0 = unpaced): trigger k
                         # waits DMA k-pace_in, so completion sems fire
                         # promptly after their data instead of lagging a
                         # whole queued DMA behind
        lead_pair=False, # load fp8 pair0-h0 as its own small DMA right
                         # after x01 so the first real matmul's wait fires
                         # before the big half-DMA's laggy semaphore
    )
    cfg = {**defaults, **(cfg or {})}

    f32 = mybir.dt.float32
    f16 = mybir.dt.float16
    f8 = mybir.dt.float8e4
    DR = mybir.MatmulPerfMode.DoubleRow
    ET = mybir.EngineType

    S8 = cfg["fp8_sub"]
    assert S8 % 2 == 0 and 0 <= S8 <= NSUB
    S16 = NSUB - S8
    NPAIR = S8 // 2

    orig_range = bass.get_kernel_semaphore_range
    if cfg["sem_stop"]:
        stop = cfg["sem_stop"]
        bass.get_kernel_semaphore_range = lambda: range(150, stop)
    try:
        nc = bacc.Bacc(None, target_bir_lowering=False)
    finally:
        bass.get_kernel_semaphore_range = orig_range

    NB = BLOC // P  # number of 128-row batch tiles per core

    # x is host-prepacked into per-b-tile stripes already in SBUF layout
    # [bt, p, bytes] with the fp8 subtiles' bytes followed by the fp16
    # subtiles' bytes, so each b-tile's stationary operand is one contiguous
    # full-bandwidth DMA; matmuls use bitcast views into the stripe.
    X8B = S8 * P  # fp8 bytes per stripe row
    XBYTES = X8B + S16 * P * 2
    u8 = mybir.dt.uint8
    # partition-major so multi-stripe slices match the SBUF layout directly
    xtc = nc.dram_tensor("xtc", [P, NB, XBYTES], u8, kind="ExternalInput")
    mt8h = mt16h = None
    if S8:
        # [half, p, s, 512]: "pair" mode slices [h, :, 2c:2c+2, :] (~128KB
        # DMAs, 1KB rows), "halfall" mode loads a whole half [h] as one DMA
        # with 3KB-contiguous rows
        mt8h = nc.dram_tensor("mt8h", [NH, P, S8, 512], f8, kind="ExternalInput")
    if S16:
        mt16h = nc.dram_tensor("mt16h", [NH, P, S16, 512], f16, kind="ExternalInput")
    lp = nc.dram_tensor("lp", [BLOC, N], f16, kind="ExternalOutput")

    # dummy warmup operands: raw (non-tile) SBUF garbage + a dedicated PSUM
    # bank; no data deps, so the hoisted matmuls are free of semaphore waits.
    # matmul() auto-emits a paired InstLdweights, so capture the warmup
    # instructions by diffing the entry block around the emission.
    wu_src = nc.alloc_sbuf_tensor("wu_src", [P, 512], f16)
    wu_ps = nc.alloc_psum_tensor("wu_ps", [P, 512], f32)
    mainblk = nc.m.functions[0].blocks[0]
    n_before = len(mainblk.instructions)
    for _ in range(cfg["warmup"]):
        nc.tensor.matmul(
            wu_ps[:], wu_src[:, 0:P], wu_src[:], start=True, stop=True
        )
    wu_insts = list(mainblk.instructions[n_before:])

    trig = {"sync": [], "scalar": [], "pool": []}

    with tile.TileContext(nc) as tc:
        with (
            tc.tile_pool(name="resident", bufs=1) as resident,
            tc.tile_pool(name="work", bufs=cfg["bufs_work"]) as work,
            tc.tile_pool(name="psum", bufs=cfg["psum_bufs"], space="PSUM") as psum_pool,
        ):
            xtc_sb = resident.tile([P, NB, XBYTES], u8, name="xtc_sb")
            # h-major SBUF layout: chunk (c|s, h) is contiguous within each
            # partition, so its load is one >=1KB-row DMA per partition
            mt8_sb = (
                resident.tile([P, NH, S8, 512], f8, name="mt8_sb") if S8 else None
            )
            mt16_sb = (
                resident.tile([P, NH, S16, 512], f16, name="mt16_sb") if S16 else None
            )

            def x8_view(bt, c):  # DR pass c stationary operand [P, 2, 128]
                return (
                    xtc_sb[:, bt, 0:X8B]
                    .bitcast(f8)
                    .rearrange("p (s c) -> p s c", c=P)[:, 2 * c : 2 * c + 2, :]
                )

            def x16_view(bt, s):  # fp16 pass s stationary operand [P, 128]
                return (
                    xtc_sb[:, bt, X8B:XBYTES]
                    .bitcast(f16)
                    .rearrange("p (s c) -> p s c", c=P)[:, s, :]
                )

            # extra warmups that stay at the top of the PE stream: if the
            # input DMA head outlasts the hoisted warmup chain these bridge
            # the gap so HAM doesn't re-throttle
            for _ in range(cfg["warmup_tail"]):
                nc.tensor.matmul(
                    wu_ps[:], wu_src[:, 0:P], wu_src[:], start=True, stop=True
                )

            # Input loads.  The 16 DMA data movers are a pool shared across
            # queues, so concurrent queues delay each other's COMPLETIONS
            # (a chunk's semaphore only fires when its last packet drains).
            # Instead, all critical inputs go down ONE queue as a few large
            # consolidated DMAs in exact consumption order: x stripes 0+1,
            # then each column-half's means (fp8 then fp16).  The remaining
            # x stripes ride the gpsimd SWDGE queue, deferred until the
            # critical loads have landed.
            seq = [(xtc_sb[:, 0:2], xtc[:, 0:2])]
            for h in range(NH):
                if cfg["mt_chunks"] == "pair":
                    for c in range(NPAIR):
                        seq.append(
                            (
                                mt8_sb[:, h, 2 * c : 2 * c + 2, :],
                                mt8h[h, :, 2 * c : 2 * c + 2, :],
                            )
                        )
                elif h == 0 and cfg["lead_pair"]:
                    seq.append(
                        (mt8_sb[:, 0, 0:2, :], mt8h[0, :, 0:2, :])
                    )
                    seq.append(
                        (mt8_sb[:, 0, 2:S8, :], mt8h[0, :, 2:S8, :])
                    )
                else:
                    seq.append((mt8_sb[:, h], mt8h[h]))
                if S16:
                    seq.append((mt16_sb[:, h], mt16h[h]))
            flush_sc = None
            if cfg["dummy_flush"]:
                flush_sc = nc.alloc_sbuf_tensor("flush_sc", [P, 64], u8)
            last_in = None
            in_tis = []
            for dst, src in seq:
                ti = nc.sync.dma_start(dst, src)
                if cfg["pace_in"] and len(in_tis) >= cfg["pace_in"]:
                    add_dep_helper(
                        ti.ins,
                        in_tis[-cfg["pace_in"]].ins,
                        sync=True,
                        reason="bound input queue depth",
                    )
                in_tis.append(ti)
                last_in = ti
                trig["sync"].append(ti.ins)
                if flush_sc is not None:
                    # tiny follow-up DMA so the big one's completion
                    # semaphore isn't held back by queue lookahead
                    fi = nc.sync.dma_start(flush_sc[:], xtc[:, 0, 0:64])
                    trig["sync"].append(fi.ins)
            xt_eng = {"pool": nc.gpsimd, "sync": nc.sync, "scalar": nc.scalar}[
                cfg["xt_queue"]
            ]
            xt_key = {"pool": "pool", "sync": "sync", "scalar": "scalar"}[
                cfg["xt_queue"]
            ]
            for bt in range(2, NB):
                ti = xt_eng.dma_start(xtc_sb[:, bt], xtc[:, bt])
                if bt == 2 and xt_key == "pool":
                    add_dep_helper(
                        ti.ins,
                        last_in.ins,
                        sync=True,
                        reason="defer SWDGE xt behind critical HWDGE loads",
                    )
                trig[xt_key].append(ti.ins)

            # matmul pass list: DoubleRow fp8 pairs first, fp16 after
            def passes(bt):
                out = []
                for c in range(NPAIR):
                    out.append(
                        lambda h, c=c, bt=bt: (
                            x8_view(bt, c),
                            mt8_sb[:, h, 2 * c : 2 * c + 2, :],
                            DR,
                        )
                    )
                for s in range(S16):
                    out.append(
                        lambda h, s=s, bt=bt: (
                            x16_view(bt, s),
                            mt16_sb[:, h, s, :],
                            None,
                        )
                    )
                return out

            prev_mm = None

            def emit_out(bt, ot, h=None, split=True):
                # split outputs across both HWDGE queues by partition
                # halves so no single queue ever backs up at the tail
                half = P // 2
                csl = slice(0, N) if h is None else slice(h * 512, (h + 1) * 512)
                if split:
                    nc.sync.dma_start(lp[bt * P : bt * P + half, csl], ot[0:half])
                    nc.scalar.dma_start(
                        lp[bt * P + half : (bt + 1) * P, csl], ot[half:P]
                    )
                else:
                    [nc.sync, nc.scalar][bt % 2].dma_start(
                        lp[bt * P : (bt + 1) * P, csl], ot[0:P]
                    )

            # b-tiles 0+1 run as one fused pass-major group: the mt chunk
            # demand rate is halved (one chunk per 2 matmuls) while the
            # input DMAs are still landing, so the PE never gaps (a gap
            # >~1us makes HAM re-throttle the clock to 4/8 mid-stream)
            ps01 = [
                psum_pool.tile([P, N], f32, tag="ps", name=f"ps0{b}")
                for b in range(2)
            ]
            pl01 = [passes(0), passes(1)]
            npass = len(pl01[0])
            for h in range(NH):
                for i in range(npass):
                    for b in range(2):
                        lhsT, rhs, pm = pl01[b][i](h)
                        mmi = nc.tensor.matmul(
                            ps01[b][:, h * 512 : (h + 1) * 512],
                            lhsT,
                            rhs,
                            start=(i == 0),
                            stop=(i == npass - 1),
                            perf_mode=pm,
                        )
                        if i == npass - 1 and b == 1:
                            prev_mm = mmi
            for b in range(2):
                ot = work.tile([P, N], f16, tag="ot", name=f"ot0{b}")
                nc.vector.tensor_copy(ot, ps01[b])
                emit_out(b, ot, split=cfg["out_split"])

            for bt in range(2, NB):
                stag = cfg["tail_stagger"] and bt == NB - 1
                if stag:
                    # separate per-h psum tiles so h0's cast (a tile-level
                    # read) doesn't falsely serialize against h1's matmuls
                    ps_h = [
                        psum_pool.tile([P, 512], f32, tag="ps", name=f"pst{h}")
                        for h in range(NH)
                    ]
                else:
                    ps = psum_pool.tile([P, N], f32, tag="ps")
                pl = passes(bt)
                npass = len(pl)
                # h-chunk-major on the last tile so h0's cast+DMA overlap
                # h1's matmuls; optionally pass-major mid-stream (fewer
                # accumulation-group boundary stalls).  NOTE: pass order
                # within an accumulation group must keep all DoubleRow
                # passes before all fp16 passes — mixing them
                # non-monotonically corrupts the accumulation.
                if stag or cfg["mid_order"] == "h":
                    his = [(h, i) for h in range(NH) for i in range(npass)]
                else:
                    his = [(h, i) for i in range(npass) for h in range(NH)]
                for h, i in his:
                    if True:
                        lhsT, rhs, pm = pl[i](h)
                        pdst = (
                            ps_h[h][:, :] if stag else ps[:, h * 512 : (h + 1) * 512]
                        )
                        mmi = nc.tensor.matmul(
                            pdst,
                            lhsT,
                            rhs,
                            start=(i == 0),
                            stop=(i == npass - 1),
                            perf_mode=pm,
                        )
                        # serialize b-tile groups on PE so each group
                        # completes (and its copy-out starts) ASAP
                        if h == 0 and i == 0 and prev_mm is not None:
                            add_dep_helper(
                                mmi.ins,
                                prev_mm.ins,
                                sync=False,
                                reason="group-sequential PE order",
                            )
                        if i == npass - 1:
                            prev_mm = mmi
                            if stag:
                                ot = work.tile([P, 512], f16, tag="ot2")
                                nc.vector.tensor_copy(ot, ps_h[h][:, :])
                                emit_out(bt, ot, h=h, split=cfg["tail_split"])
                if not stag:
                    ot = work.tile([P, N], f16, tag="ot")
                    nc.vector.tensor_copy(ot, ps)
                    emit_out(bt, ot, split=cfg["out_split"])

    if cfg["hoist"]:
        _hoist(nc, mybir, trig, wu_insts)

    nc.compile()
    return nc


def _hoist(nc, mybir, trig, wu_insts):
    """Move the input DMA triggers and warmup matmuls from the tile-context
    block into the entry block's all-engine-barrier arrive slots.

    The entry barrier is, per engine, (InstDrain[arrive], InstEventSemaphore
    [wait-release]); instructions placed between the two run right after that
    engine's fixed walrus preamble without delaying any other engine.  The
    gpsimd x-stripe triggers go after the barrier release (gpsimd is the
    barrier master, so anything before its release EventSemaphore would
    stall every engine)."""
    ET = mybir.EngineType
    f = nc.m.functions[0]
    b0, b1 = f.blocks[0], f.blocks[1]

    moved = {
        ET.SP: list(trig["sync"]),
        ET.Activation: list(trig["scalar"]),
        ET.PE: list(wu_insts),
        ET.Pool: list(trig["pool"]),
    }
    # warmups emitted pre-tile-context already live in b0 (after the
    # barrier); everything else is in b1
    move_ids = {id(x) for insts in moved.values() for x in insts}
    b0.instructions = [x for x in b0.instructions if id(x) not in move_ids]
    b1.instructions = [x for x in b1.instructions if id(x) not in move_ids]

    def arrive_slot(eng):
        for i, ins in enumerate(b0.instructions):
            if isinstance(ins, mybir.InstDrain) and ins.engine == eng:
                return i + 1
        raise RuntimeError(f"no barrier drain found for {eng}")

    def after_release():
        last = None
        for i, ins in enumerate(b0.instructions):
            if isinstance(ins, mybir.InstEventSemaphore) and ins.engine == ET.Pool:
                last = i
        assert last is not None
        return last + 1

    for eng in (ET.SP, ET.Activation, ET.PE):
        if moved[eng]:
            pos = arrive_slot(eng)
            b0.instructions[pos:pos] = moved[eng]
    if moved[ET.Pool]:
        pos = after_release()
        b0.instructions[pos:pos] = moved[ET.Pool]


def _host_prep(x, means, bandwidths, weights, priors, fp8_sub):
    """Pack transposed GEMM operands; compute host-side affine terms."""
    import ml_dtypes

    x = np.asarray(x, dtype=np.float32)
    means = np.asarray(means, dtype=np.float32)

    bw = np.clip(np.asarray(bandwidths, dtype=np.float64), 0.001, 100.0)
    a = 1.0 / bw
    m_sq = np.einsum(
        "nd,nd->n", means.astype(np.float64), means.astype(np.float64)
    )
    w = np.asarray(weights, dtype=np.float64).reshape(C, K)
    log_w = (
        w
        - np.log(np.exp(w - w.max(1, keepdims=True)).sum(1, keepdims=True))
        - w.max(1, keepdims=True)
    ).reshape(N)
    pr = np.asarray(priors, dtype=np.float64)
    log_pri = pr - (np.log(np.exp(pr - pr.max()).sum()) + pr.max())
    cvec = (
        -0.5 * (D * LOG_2PI + D * np.log(bw) + m_sq * a)
        + log_w
        + np.repeat(log_pri, K)
    )
    ah = -0.5 * a
    xsq = np.einsum("bd,bd->b", x.astype(np.float64), x.astype(np.float64))

    # pack x into per-core, per-b-tile stripes [core, bt, p(row), bytes]:
    # fp8 subtile bytes then fp16 subtile bytes, matching the device bitcast
    nbt = BLOC // P
    ds = fp8_sub * P
    xt_t = x.T  # [D, B]
    mt_t = means.T * a  # [D, N]

    def pack_x(arr, dt):  # arr [d, B] -> [core, p(row), bt, sub*col] bytes
        sub = arr.shape[0] // P
        packed = np.ascontiguousarray(
            arr.astype(dt).reshape(sub, P, NCORES, nbt, P).transpose(2, 1, 3, 0, 4)
        )
        return packed.reshape(NCORES, P, nbt, -1).view(np.uint8)

    chunks = []
    if fp8_sub:
        chunks.append(pack_x(xt_t[:ds], ml_dtypes.float8_e4m3))
    if ds < D:
        chunks.append(pack_x(xt_t[ds:], np.float16))
    parts = {"xtc": np.concatenate(chunks, axis=3)}
    if fp8_sub:
        # [h, p, s, j]: element = mt[s*P+p, h*512+j]
        m8 = mt_t[:ds].astype(ml_dtypes.float8_e4m3)
        m8 = m8.reshape(fp8_sub, P, 2, 512).transpose(2, 1, 0, 3)
        parts["mt8h"] = np.ascontiguousarray(m8)
    if ds < D:
        m16 = mt_t[ds:].astype(np.float16)
        m16 = m16.reshape((D - ds) // P, P, 2, 512).transpose(2, 1, 0, 3)
        parts["mt16h"] = np.ascontiguousarray(m16)
    return parts, cvec, ah, xsq


def _host_finish(lp, cvec, ah, xsq):
    """lp: [B, N] fp16 device GEMM result -> [B, C] float32 log-mixture."""
    logp = lp.astype(np.float32)
    logp += cvec.astype(np.float32)[None, :]
    logp += xsq.astype(np.float32)[:, None] * ah.astype(np.float32)[None, :]
    v = logp.reshape(B, C, K)
    m = v.max(2)
    lse1 = m + np.log(np.exp(v - m[:, :, None]).sum(2, dtype=np.float32))
    z = lse1.max(1, keepdims=True)
    out = lse1 - (
        z + np.log(np.exp(lse1 - z).sum(1, keepdims=True, dtype=np.float32))
    )
    return out.astype(np.float32)


def _run(x, means, bandwidths, weights, priors, trace=False, cfg=None):
    from concourse.bass_utils import run_bass_kernel_spmd

    key = tuple(sorted((cfg or {}).items()))
    if key not in _CACHE:
        try:
            _CACHE[key] = _build_nc(cfg)
        except Exception:
            # the entry-block hoisting surgery is purely a perf
            # transformation; fall back to the plain schedule if the
            # framework's block layout ever changes underneath it
            if (cfg or {}).get("hoist", True):
                _CACHE[key] = _build_nc({**(cfg or {}), "hoist": False})
            else:
                raise
    nc = _CACHE[key]
    fp8_sub = (cfg or {}).get("fp8_sub", 6)

    parts, cvec, ah, xsq = _host_prep(
        x, means, bandwidths, weights, priors, fp8_sub
    )
    in_maps = [
        {
            k: np.ascontiguousarray(v[i]) if k.startswith("xt") else v
            for k, v in parts.items()
        }
        for i in range(NCORES)
    ]
    res = run_bass_kernel_spmd(nc, in_maps, core_ids=list(range(NCORES)), trace=trace)
    lp = np.concatenate([r["lp"] for r in res.results], axis=0)
    out = _host_finish(lp, cvec, ah, xsq)
    return out, res


def kernel(x, means, bandwidths, weights, priors):
    out, _ = _run(x, means, bandwidths, weights, priors, trace=False)
    return out


# revision 66
# speedup vs baseline: 1.0984x; 1.0406x over previous
"""Trainium2 Bass kernel for spherical deep GMM classifier (DGMMC).

Reference computation (B=8192, D=1024, C=128 classes, K=8 comps, N=C*K=1024):
    bw = clip(bandwidths, 1e-3, 100); a = 1/bw
    log_prob[b,n] = -0.5*(D*log(2pi) + D*log(bw[n]) + sq_dist[b,n]/bw[n])
    log_prob += log_softmax(weights.reshape(C,K),1).reshape(N)
    lse1[b,c]  = LSE_k(log_prob[b,c*K+k]) + log_softmax(priors)[c]
    out[b,c]   = lse1[b,c] - LSE_c(lse1[b,c])

Strategy: data-parallel over batch across 8 cores.  The device computes the
single dominant GEMM  dot[b,n] = x[b,:] @ (means * (1/bw))[n,:].T  and writes
dot back as fp16; 6 of the 8 128-dim contraction subtiles run as fp8-e4m3
DoubleRow matmuls, the other 2 as fp16 (full fp8 fails the 2e-2 gate:
2.07e-2 measured offline).  Everything else is O(B*N) or smaller and runs
on the host around the device call.

Perf structure (vs. the naive schedule):
  - Input DMA triggers are hoisted into the kernel-entry all-engine
    barrier's "arrive" slots on SP/ACT, so HBM traffic starts ~2.5us
    earlier (right after the fixed walrus engine preamble instead of after
    the const-init barrier).
  - A chain of dummy "warmup" matmuls reading uninitialized SBUF is
    hoisted the same way onto the PE queue: the HAM clock-ramp (~4us of
    sustained PE activity to reach 8/8) runs concurrently with the input
    load instead of serializing in front of the real stream.
  - Each means chunk is split into its two 512-column halves with one half
    on each HWDGE queue, so both queues feed the first b-tile's pass sweep
    concurrently (~330 GB/s aggregate).
  - x stripes ride the gpsimd software-DGE queue, keeping the two HWDGE
    queues free for means + output traffic.
  - The last b-tile is processed h-chunk-major and its output is cast +
    DMA'd per 512-column chunk, split across both queues by partition
    halves, to minimize the post-stream tail.
"""

import math

import numpy as np

B, D, C, K = 8192, 1024, 128, 8
N = C * K
NCORES = 8
BLOC = B // NCORES  # rows per core
P = 128
NSUB = D // P  # 128-row contraction subtiles
NH = N // 512
LOG_2PI = math.log(2.0 * math.pi)

_CACHE: dict = {}


def _build_nc(cfg=None):
    import concourse.bacc as bacc
    import concourse.bass as bass
    import concourse.mybir as mybir
    import concourse.tile as tile
    from concourse.tile import add_dep_helper

    defaults = dict(
        fp8_sub=6,       # leading 128-dim subtiles done in e4m3 DoubleRow
        warmup=12,       # dummy matmuls (garbage SBUF src) hoisted into the
                         # entry barrier to ramp the HAM clock to 8/8 while
                         # the input DMA head is in flight
        warmup_tail=1,   # non-hoisted warmups at stream top as a cushion so
                         # the PE never gaps between warmup chain and data
        hoist=True,      # move triggers/warmups into block-0 arrive slots
        xt_queue="pool", # "pool": x stripes on gpsimd SWDGE; "hw": on HWDGE
        bufs_work=4,
        psum_bufs=3,     # 3x [P,N] fp32 = 6 banks; wu_ps takes a 7th
        sem_stop=200,    # shrink kernel semaphore range (reset-sweep length)
        tail_stagger=True,
        tail_split=True, # split last-tile output DMAs across both queues
        mt_chunks="halfall",  # "pair": ~128KB mt DMAs; "halfall": one per half
        dummy_flush=False,  # tiny DMA after each mt load to flush its sem
        mid_order="h",   # bt>=2 matmul order: "h" chunk-major / "pass" major
        out_split=False,  # single alternating-queue DMA per non-tail output
        pace_in=0,       # input-queue depth limit (0 = unpaced): trigger k
                         # waits DMA k-pace_in, so completion sems fire
                         # promptly after their data instead of lagging a
                         # whole queued DMA behind
        lead_pair=False, # load fp8 pair0-h0 as its own small DMA right
                         # after x01 so the first real matmul's wait fires
                         # before the big half-DMA's laggy semaphore
        xt_anchor=1,     # input-seq index whose completion releases the
                         # deferred gpsimd x-stripe loads
    )
    cfg = {**defaults, **(cfg or {})}

    f32 = mybir.dt.float32
    f16 = mybir.dt.float16
    f8 = mybir.dt.float8e4
    DR = mybir.MatmulPerfMode.DoubleRow
    ET = mybir.EngineType

    S8 = cfg["fp8_sub"]
    assert S8 % 2 == 0 and 0 <= S8 <= NSUB
    S16 = NSUB - S8
    NPAIR = S8 // 2

    orig_range = bass.get_kernel_semaphore_range
    if cfg["sem_stop"]:
        stop = cfg["sem_stop"]
        bass.get_kernel_semaphore_range = lambda: range(150, stop)
    try:
        nc = bacc.Bacc(None, target_bir_lowering=False)
    finally:
        bass.get_kernel_semaphore_range = orig_range

    NB = BLOC // P  # number of 128-row batch tiles per core

    # x is host-prepacked into per-b-tile stripes already in SBUF layout
    # [bt, p, bytes] with the fp8 subtiles' bytes followed by the fp16
    # subtiles' bytes, so each b-tile's stationary operand is one contiguous
    # full-bandwidth DMA; matmuls use bitcast views into the stripe.
    X8B = S8 * P  # fp8 bytes per stripe row
    XBYTES = X8B + S16 * P * 2
    u8 = mybir.dt.uint8
    # partition-major so multi-stripe slices match the SBUF layout directly
    xtc = nc.dram_tensor("xtc", [P, NB, XBYTES], u8, kind="ExternalInput")
    mt8h = mt16h = None
    if S8:
        # [half, p, s, 512]: "pair" mode slices [h, :, 2c:2c+2, :] (~128KB
        # DMAs, 1KB rows), "halfall" mode loads a whole half [h] as one DMA
        # with 3KB-contiguous rows
        mt8h = nc.dram_tensor("mt8h", [NH, P, S8, 512], f8, kind="ExternalInput")
    if S16:
        mt16h = nc.dram_tensor("mt16h", [NH, P, S16, 512], f16, kind="ExternalInput")
    lp = nc.dram_tensor("lp", [BLOC, N], f16, kind="ExternalOutput")

    # dummy warmup operands: raw (non-tile) SBUF garbage + a dedicated PSUM
    # bank; no data deps, so the hoisted matmuls are free of semaphore waits.
    # matmul() auto-emits a paired InstLdweights, so capture the warmup
    # instructions by diffing the entry block around the emission.
    wu_src = nc.alloc_sbuf_tensor("wu_src", [P, 512], f16)
    wu_ps = nc.alloc_psum_tensor("wu_ps", [P, 512], f32)
    mainblk = nc.m.functions[0].blocks[0]
    n_before = len(mainblk.instructions)
    for _ in range(cfg["warmup"]):
        nc.tensor.matmul(
            wu_ps[:], wu_src[:, 0:P], wu_src[:], start=True, stop=True
        )
    wu_insts = list(mainblk.instructions[n_before:])

    trig = {"sync": [], "scalar": [], "pool": []}

    with tile.TileContext(nc) as tc:
        with (
            tc.tile_pool(name="resident", bufs=1) as resident,
            tc.tile_pool(name="work", bufs=cfg["bufs_work"]) as work,
            tc.tile_pool(name="psum", bufs=cfg["psum_bufs"], space="PSUM") as psum_pool,
        ):
            xtc_sb = resident.tile([P, NB, XBYTES], u8, name="xtc_sb")
            # h-major SBUF layout: chunk (c|s, h) is contiguous within each
            # partition, so its load is one >=1KB-row DMA per partition
            mt8_sb = (
                resident.tile([P, NH, S8, 512], f8, name="mt8_sb") if S8 else None
            )
            mt16_sb = (
                resident.tile([P, NH, S16, 512], f16, name="mt16_sb") if S16 else None
            )

            def x8_view(bt, c):  # DR pass c stationary operand [P, 2, 128]
                return (
                    xtc_sb[:, bt, 0:X8B]
                    .bitcast(f8)
                    .rearrange("p (s c) -> p s c", c=P)[:, 2 * c : 2 * c + 2, :]
                )

            def x16_view(bt, s):  # fp16 pass s stationary operand [P, 128]
                return (
                    xtc_sb[:, bt, X8B:XBYTES]
                    .bitcast(f16)
                    .rearrange("p (s c) -> p s c", c=P)[:, s, :]
                )

            # extra warmups that stay at the top of the PE stream: if the
            # input DMA head outlasts the hoisted warmup chain these bridge
            # the gap so HAM doesn't re-throttle
            for _ in range(cfg["warmup_tail"]):
                nc.tensor.matmul(
                    wu_ps[:], wu_src[:, 0:P], wu_src[:], start=True, stop=True
                )

            # Input loads.  The 16 DMA data movers are a pool shared across
            # queues, so concurrent queues delay each other's COMPLETIONS
            # (a chunk's semaphore only fires when its last packet drains).
            # Instead, all critical inputs go down ONE queue as a few large
            # consolidated DMAs in exact consumption order: x stripes 0+1,
            # then each column-half's means (fp8 then fp16).  The remaining
            # x stripes ride the gpsimd SWDGE queue, deferred until the
            # critical loads have landed.
            seq = [(xtc_sb[:, 0:2], xtc[:, 0:2])]
            for h in range(NH):
                if cfg["mt_chunks"] == "pair":
                    for c in range(NPAIR):
                        seq.append(
                            (
                                mt8_sb[:, h, 2 * c : 2 * c + 2, :],
                                mt8h[h, :, 2 * c : 2 * c + 2, :],
                            )
                        )
                elif h == 0 and cfg["lead_pair"]:
                    seq.append(
                        (mt8_sb[:, 0, 0:2, :], mt8h[0, :, 0:2, :])
                    )
                    seq.append(
                        (mt8_sb[:, 0, 2:S8, :], mt8h[0, :, 2:S8, :])
                    )
                else:
                    seq.append((mt8_sb[:, h], mt8h[h]))
                if S16:
                    seq.append((mt16_sb[:, h], mt16h[h]))
            flush_sc = None
            if cfg["dummy_flush"]:
                flush_sc = nc.alloc_sbuf_tensor("flush_sc", [P, 64], u8)
            last_in = None
            in_tis = []
            for dst, src in seq:
                ti = nc.sync.dma_start(dst, src)
                if cfg["pace_in"] and len(in_tis) >= cfg["pace_in"]:
                    add_dep_helper(
                        ti.ins,
                        in_tis[-cfg["pace_in"]].ins,
                        sync=True,
                        reason="bound input queue depth",
                    )
                in_tis.append(ti)
                last_in = ti
                trig["sync"].append(ti.ins)
                if flush_sc is not None:
                    # tiny follow-up DMA so the big one's completion
                    # semaphore isn't held back by queue lookahead
                    fi = nc.sync.dma_start(flush_sc[:], xtc[:, 0, 0:64])
                    trig["sync"].append(fi.ins)
            xt_eng = {"pool": nc.gpsimd, "sync": nc.sync, "scalar": nc.scalar}[
                cfg["xt_queue"]
            ]
            xt_key = {"pool": "pool", "sync": "sync", "scalar": "scalar"}[
                cfg["xt_queue"]
            ]
            # Anchor the deferred stripes on a mid-sequence DMA: completion
            # semaphores lag their data by roughly one queued DMA, so the
            # 3rd DMA's sem fires just as the whole input chain drains.
            anchor = in_tis[min(cfg["xt_anchor"], len(in_tis) - 1)]
            for bt in range(2, NB):
                ti = xt_eng.dma_start(xtc_sb[:, bt], xtc[:, bt])
                if bt == 2 and xt_key == "pool":
                    add_dep_helper(
                        ti.ins,
                        anchor.ins,
                        sync=True,
                        reason="defer SWDGE xt behind critical HWDGE loads",
                    )
                trig[xt_key].append(ti.ins)

            # matmul pass list: DoubleRow fp8 pairs first, fp16 after
            def passes(bt):
                out = []
                for c in range(NPAIR):
                    out.append(
                        lambda h, c=c, bt=bt: (
                            x8_view(bt, c),
                            mt8_sb[:, h, 2 * c : 2 * c + 2, :],
                            DR,
                        )
                    )
                for s in range(S16):
                    out.append(
                        lambda h, s=s, bt=bt: (
                            x16_view(bt, s),
                            mt16_sb[:, h, s, :],
                            None,
                        )
                    )
                return out

            prev_mm = None

            def emit_out(bt, ot, h=None, split=True):
                # split outputs across both HWDGE queues by partition
                # halves so no single queue ever backs up at the tail
                half = P // 2
                csl = slice(0, N) if h is None else slice(h * 512, (h + 1) * 512)
                if split:
                    nc.sync.dma_start(lp[bt * P : bt * P + half, csl], ot[0:half])
                    nc.scalar.dma_start(
                        lp[bt * P + half : (bt + 1) * P, csl], ot[half:P]
                    )
                else:
                    [nc.sync, nc.scalar][bt % 2].dma_start(
                        lp[bt * P : (bt + 1) * P, csl], ot[0:P]
                    )

            # b-tiles 0+1 run as one fused pass-major group: the mt chunk
            # demand rate is halved (one chunk per 2 matmuls) while the
            # input DMAs are still landing, so the PE never gaps (a gap
            # >~1us makes HAM re-throttle the clock to 4/8 mid-stream)
            ps01 = [
                psum_pool.tile([P, N], f32, tag="ps", name=f"ps0{b}")
                for b in range(2)
            ]
            pl01 = [passes(0), passes(1)]
            npass = len(pl01[0])
            for h in range(NH):
                for i in range(npass):
                    for b in range(2):
                        lhsT, rhs, pm = pl01[b][i](h)
                        mmi = nc.tensor.matmul(
                            ps01[b][:, h * 512 : (h + 1) * 512],
                            lhsT,
                            rhs,
                            start=(i == 0),
                            stop=(i == npass - 1),
                            perf_mode=pm,
                        )
                        if i == npass - 1 and b == 1:
                            prev_mm = mmi
            for b in range(2):
                ot = work.tile([P, N], f16, tag="ot", name=f"ot0{b}")
                nc.vector.tensor_copy(ot, ps01[b])
                emit_out(b, ot, split=cfg["out_split"])

            for bt in range(2, NB):
                stag = cfg["tail_stagger"] and bt == NB - 1
                if stag:
                    # separate per-h psum tiles so h0's cast (a tile-level
                    # read) doesn't falsely serialize against h1's matmuls
                    ps_h = [
                        psum_pool.tile([P, 512], f32, tag="ps", name=f"pst{h}")
                        for h in range(NH)
                    ]
                else:
                    ps = psum_pool.tile([P, N], f32, tag="ps")
                pl = passes(bt)
                npass = len(pl)
                # h-chunk-major on the last tile so h0's cast+DMA overlap
                # h1's matmuls; optionally pass-major mid-stream (fewer
                # accumulation-group boundary stalls).  NOTE: pass order
                # within an accumulation group must keep all DoubleRow
                # passes before all fp16 passes — mixing them
                # non-monotonically corrupts the accumulation.
                if stag or cfg["mid_order"] == "h":
                    his = [(h, i) for h in range(NH) for i in range(npass)]
                else:
                    his = [(h, i) for i in range(npass) for h in range(NH)]
                for h, i in his:
                    if True:
                        lhsT, rhs, pm = pl[i](h)
                        pdst = (
                            ps_h[h][:, :] if stag else ps[:, h * 512 : (h + 1) * 512]
                        )
                        mmi = nc.tensor.matmul(
                            pdst,
                            lhsT,
                            rhs,
                            start=(i == 0),
                            stop=(i == npass - 1),
                            perf_mode=pm,
                        )
                        # serialize b-tile groups on PE so each group
                        # completes (and its copy-out starts) ASAP
                        if h == 0 and i == 0 and prev_mm is not None:
                            add_dep_helper(
                                mmi.ins,
                                prev_mm.ins,
                                sync=False,
                                reason="group-sequential PE order",
                            )
                        if i == npass - 1:
                            prev_mm = mmi
                            if stag:
                                ot = work.tile([P, 512], f16, tag="ot2")
                                nc.vector.tensor_copy(ot, ps_h[h][:, :])
                                emit_out(bt, ot, h=h, split=cfg["tail_split"])
                if not stag:
                    ot = work.tile([P, N], f16, tag="ot")
                    nc.vector.tensor_copy(ot, ps)
                    emit_out(bt, ot, split=cfg["out_split"])

    if cfg["hoist"]:
        _hoist(nc, mybir, trig, wu_insts)

    nc.compile()
    return nc


def _hoist(nc, mybir, trig, wu_insts):
    """Move the input DMA triggers and warmup matmuls from the tile-context
    block into the entry block's all-engine-barrier arrive slots.

    The entry barrier is, per engine, (InstDrain[arrive], InstEventSemaphore
    [wait-release]); instructions placed between the two run right after that
    engine's fixed walrus preamble without delaying any other engine.  The
    gpsimd x-stripe triggers go after the barrier release (gpsimd is the
    barrier master, so anything before its release EventSemaphore would
    stall every engine)."""
    ET = mybir.EngineType
    f = nc.m.functions[0]
    b0, b1 = f.blocks[0], f.blocks[1]

    moved = {
        ET.SP: list(trig["sync"]),
        ET.Activation: list(trig["scalar"]),
        ET.PE: list(wu_insts),
        ET.Pool: list(trig["pool"]),
    }
    # warmups emitted pre-tile-context already live in b0 (after the
    # barrier); everything else is in b1
    move_ids = {id(x) for insts in moved.values() for x in insts}
    b0.instructions = [x for x in b0.instructions if id(x) not in move_ids]
    b1.instructions = [x for x in b1.instructions if id(x) not in move_ids]

    def arrive_slot(eng):
        for i, ins in enumerate(b0.instructions):
            if isinstance(ins, mybir.InstDrain) and ins.engine == eng:
                return i + 1
        raise RuntimeError(f"no barrier drain found for {eng}")

    def after_release():
        last = None
        for i, ins in enumerate(b0.instructions):
            if isinstance(ins, mybir.InstEventSemaphore) and ins.engine == ET.Pool:
                last = i
        assert last is not None
        return last + 1

    for eng in (ET.SP, ET.Activation, ET.PE):
        if moved[eng]:
            pos = arrive_slot(eng)
            b0.instructions[pos:pos] = moved[eng]
    if moved[ET.Pool]:
        pos = after_release()
        b0.instructions[pos:pos] = moved[ET.Pool]


def _host_prep(x, means, bandwidths, weights, priors, fp8_sub):
    """Pack transposed GEMM operands; compute host-side affine terms."""
    import ml_dtypes

    x = np.asarray(x, dtype=np.float32)
    means = np.asarray(means, dtype=np.float32)

    bw = np.clip(np.asarray(bandwidths, dtype=np.float64), 0.001, 100.0)
    a = 1.0 / bw
    m_sq = np.einsum(
        "nd,nd->n", means.astype(np.float64), means.astype(np.float64)
    )
    w = np.asarray(weights, dtype=np.float64).reshape(C, K)
    log_w = (
        w
        - np.log(np.exp(w - w.max(1, keepdims=True)).sum(1, keepdims=True))
        - w.max(1, keepdims=True)
    ).reshape(N)
    pr = np.asarray(priors, dtype=np.float64)
    log_pri = pr - (np.log(np.exp(pr - pr.max()).sum()) + pr.max())
    cvec = (
        -0.5 * (D * LOG_2PI + D * np.log(bw) + m_sq * a)
        + log_w
        + np.repeat(log_pri, K)
    )
    ah = -0.5 * a
    xsq = np.einsum("bd,bd->b", x.astype(np.float64), x.astype(np.float64))

    # pack x into per-core, per-b-tile stripes [core, bt, p(row), bytes]:
    # fp8 subtile bytes then fp16 subtile bytes, matching the device bitcast
    nbt = BLOC // P
    ds = fp8_sub * P
    xt_t = x.T  # [D, B]
    mt_t = means.T * a  # [D, N]

    def pack_x(arr, dt):  # arr [d, B] -> [core, p(row), bt, sub*col] bytes
        sub = arr.shape[0] // P
        packed = np.ascontiguousarray(
            arr.astype(dt).reshape(sub, P, NCORES, nbt, P).transpose(2, 1, 3, 0, 4)
        )
        return packed.reshape(NCORES, P, nbt, -1).view(np.uint8)

    chunks = []
    if fp8_sub:
        chunks.append(pack_x(xt_t[:ds], ml_dtypes.float8_e4m3))
    if ds < D:
        chunks.append(pack_x(xt_t[ds:], np.float16))
    parts = {"xtc": np.concatenate(chunks, axis=3)}
    if fp8_sub:
        # [h, p, s, j]: element = mt[s*P+p, h*512+j]
        m8 = mt_t[:ds].astype(ml_dtypes.float8_e4m3)
        m8 = m8.reshape(fp8_sub, P, 2, 512).transpose(2, 1, 0, 3)
        parts["mt8h"] = np.ascontiguousarray(m8)
    if ds < D:
        m16 = mt_t[ds:].astype(np.float16)
        m16 = m16.reshape((D - ds) // P, P, 2, 512).transpose(2, 1, 0, 3)
        parts["mt16h"] = np.ascontiguousarray(m16)
    return parts, cvec, ah, xsq


def _host_finish(lp, cvec, ah, xsq):
    """lp: [B, N] fp16 device GEMM result -> [B, C] float32 log-mixture."""
    logp = lp.astype(np.float32)
    logp += cvec.astype(np.float32)[None, :]
    logp += xsq.astype(np.float32)[:, None] * ah.astype(np.float32)[None, :]
    v = logp.reshape(B, C, K)
    m = v.max(2)
    lse1 = m + np.log(np.exp(v - m[:, :, None]).sum(2, dtype=np.float32))
    z = lse1.max(1, keepdims=True)
    out = lse1 - (
        z + np.log(np.exp(lse1 - z).sum(1, keepdims=True, dtype=np.float32))
    )
    return out.astype(np.float32)


def _run(x, means, bandwidths, weights, priors, trace=False, cfg=None):
    from concourse.bass_utils import run_bass_kernel_spmd

    key = tuple(sorted((cfg or {}).items()))
    if key not in _CACHE:
        try:
            _CACHE[key] = _build_nc(cfg)
        except Exception:
            # the entry-block hoisting surgery is purely a perf
            # transformation; fall back to the plain schedule if the
            # framework's block layout ever changes underneath it
            if (cfg or {}).get("hoist", True):
                _CACHE[key] = _build_nc({**(cfg or {}), "hoist": False})
            else:
                raise
    nc = _CACHE[key]
    fp8_sub = (cfg or {}).get("fp8_sub", 6)

    parts, cvec, ah, xsq = _host_prep(
        x, means, bandwidths, weights, priors, fp8_sub
    )
    in_maps = [
        {
            k: np.ascontiguousarray(v[i]) if k.startswith("xt") else v
            for k, v in parts.items()
        }
        for i in range(NCORES)
    ]
    res = run_bass_kernel_spmd(nc, in_maps, core_ids=list(range(NCORES)), trace=trace)
    lp = np.concatenate([r["lp"] for r in res.results], axis=0)
    out = _host_finish(lp, cvec, ah, xsq)
    return out, res


def kernel(x, means, bandwidths, weights, priors):
    out, _ = _run(x, means, bandwidths, weights, priors, trace=False)
    return out
